# revision 21
# baseline (speedup 1.0000x reference)
"""Trainium2 Bass kernel for nn_MixedMlp (soft-mixture MoE MLP).

Math (per batch row b):
    cn = LayerNorm(c); x = [z, cn]
    coeff = softmax(gateMLP(x))                       # [E]
    l0 = elu(sum_e coeff_e (x @ w0_e + b0_e))
    l1 = elu(sum_e coeff_e ([z, l0] @ w1_e + b1_e))
    out = sum_e coeff_e ([z, l1] @ w2_e + b2_e)

Kernel strategy (8 cores, data-parallel over B=8192):
  * Activations feature-major ([features, batch]); every layer is one
    PSUM-accumulated GEMM contracting coeff-scaled inputs:
    out^T = sum_e W_e^T (coeff_e . X^T).  bfloat16 on-chip.
  * LayerNorm gamma/beta folded into W0c/g0c and biases host-side; LN
    stats computed on the Scalar engine via activation accum_out
    (sum x, sum x^2), freeing the DVE.
  * ELU as s = elu(x)+1 = relu(x) + min(exp(x), 1), -1 folded into next
    bias; exp on Scalar, relu on Vector (gate) or Scalar (expert phase),
    combine on DVE.
  * Softmax sum-of-exps lands on 8 partitions via a ones[8,8] matmul ->
    8-lane reciprocal -> one multiply.
  * Gate output processed in four 256-col quarters: each quarter's coeffs
    are staged to a contiguous DRAM block and broadcast back with
    4KB-contiguous-source stride-0 DMAs (fast descriptors), so expert
    layers start as soon as the first quarter's broadcast lands.
  * Expert layers per 512-col half; scaled-input products split between
    DVE and GpSimd; h1's softmax/broadcast chain is emitted between L0-h0's
    bias and product matmuls to keep the PE busy.
  * Activation-function table pinned once (natural_log_exp set) at t=0.
"""

import numpy as np
import ml_dtypes
from contextlib import ExitStack

import concourse.bass as bass
import concourse.bacc as bacc
import concourse.tile as tile
import concourse.mybir as mybir
from concourse import bass_utils
from concourse.bass import AP

F32 = mybir.dt.float32
BF16 = mybir.dt.bfloat16
AF = mybir.ActivationFunctionType
OP = mybir.AluOpType
NP_BF16 = ml_dtypes.bfloat16

N_CORES = 8
B = 8192
R = B // N_CORES          # rows per core = 1024
LATENT, CIN, HID, ACTD, E, GH = 32, 128, 256, 16, 8, 128
IN0, INTER = LATENT + CIN, HID + LATENT
LN_EPS = 1e-5
BG = 512                  # half width
BQ = 256                  # quarter width
NCH = R // 128            # 8 b-chunks per core

_GCOLS = [("g0z", 128), ("g0c", 128), ("g1w", 128), ("g2w", 8),
          ("b01", 512), ("on88", 8), ("i16", 128), ("ohe", 1024),
          ("sel8", 256)]
_WCOLS = [("w0z", 512), ("w0c", 2048), ("w1z", 512), ("w1h", 4096),
          ("w2s", 384), ("s2", 16)]
_GOFF, _WOFF = {}, {}
_o = 0
for _n, _c in _GCOLS:
    _GOFF[_n] = _o
    _o += _c
NGATE = _o
_o = 0
for _n, _c in _WCOLS:
    _WOFF[_n] = _o
    _o += _c
NWALL = _o
WSPLIT = _WOFF["w1z"]     # wall_a = w0 weights, wall_b = w1/w2 weights

ACT_SET_LN_EXP = 6        # natural_log_exp_and_others in act_info.json
N_GP0 = 3                 # scaled inputs on gpsimd per half, layer 0
N_GP1 = 5                 # and layer 1

_CACHE = {}


def _build_program():
    nc = bacc.Bacc("TRN2", target_bir_lowering=False, debug=False,
                   num_devices=N_CORES)

    zr_d = nc.dram_tensor("zrep", [128, R], BF16, kind="ExternalInput").ap()
    c_d = nc.dram_tensor("cperm", [128, NCH * CIN], BF16, kind="ExternalInput").ap()
    wg_d = nc.dram_tensor("wgate", [128, NGATE], BF16, kind="ExternalInput").ap()
    wall_d = nc.dram_tensor("wall", [128, NWALL], BF16, kind="ExternalInput").ap()
    ck_d = nc.dram_tensor("consts", [128, 6], F32, kind="ExternalInput").ap()
    out_d = nc.dram_tensor("out", [R, ACTD], F32, kind="ExternalOutput").ap()

    with tile.TileContext(nc) as tc, ExitStack() as ctx:
        wp = ctx.enter_context(tc.tile_pool(name="wp", bufs=1))       # weights
        big = ctx.enter_context(tc.tile_pool(name="big", bufs=1))     # persistent activations
        sp = ctx.enter_context(tc.tile_pool(name="sp", bufs=4))       # small temps
        er = ctx.enter_context(tc.tile_pool(name="er", bufs=6))       # elu temps
        sc = ctx.enter_context(tc.tile_pool(name="sc", bufs=6))       # scaled-input tiles
        pm = ctx.enter_context(tc.tile_pool(name="pm", bufs=4, space="PSUM"))   # big matmuls
        psm = ctx.enter_context(tc.tile_pool(name="psm", bufs=1, space="PSUM")) # small matmuls
        dstage = ctx.enter_context(tc.tile_pool(name="dstage", bufs=1, space="DRAM"))

        # pin the activation table (ln+exp+relu+copy+square) once, at t=0
        nc.scalar.add_instruction(mybir.InstLoadActFuncSet(
            name=nc.get_next_instruction_name(),
            act_func_set_id=ACT_SET_LN_EXP, ins=[], outs=[]))

        # ---------------- bulk loads (priority-ordered per HWDGE queue) -----
        # c pre-permuted: partition p holds rows 8p..8p+8.  on-chip batch
        # order is i = 128*r + p  <->  original row b = 8p + r.
        ctall = big.tile([128, NCH * CIN], BF16)
        for jj in range(2):
            nc.sync.dma_start(ctall[:, 4 * CIN * jj:4 * CIN * (jj + 1)],
                              c_d[:, 4 * CIN * jj:4 * CIN * (jj + 1)])
        wall = wp.tile([128, NWALL], BF16)
        nc.sync.dma_start(wall[:, 0:WSPLIT], wall_d[:, 0:WSPLIT],
                          max_dma_last_dim=4096)
        nc.sync.dma_start(wall[:, WSPLIT:], wall_d[:, WSPLIT:],
                          max_dma_last_dim=4096)
        wgate = wp.tile([128, NGATE], BF16)
        nc.scalar.dma_start(wgate[:], wg_d[:], max_dma_last_dim=4096)
        zrep = big.tile([128, R], BF16)
        nc.scalar.dma_start(zrep[:], zr_d[:])
        ckt = wp.tile([128, 6], F32)
        nc.scalar.dma_start(ckt[:], ck_d[:])

        def wsl(name, p0, pn, c0, cn_):
            if name in _GOFF:
                o = _GOFF[name]
                return wgate[p0:p0 + pn, o + c0:o + c0 + cn_]
            o = _WOFF[name]
            return wall[p0:p0 + pn, o + c0:o + c0 + cn_]
        epsc = ckt[:, 0:1]
        g0b, g1b, b2c = ckt[:, 1:2], ckt[:, 2:3], ckt[:, 3:4]
        g2b = ckt[0:8, 4:5]

        # ---------------- persistent activation tiles ----------------
        cnT = big.tile([128, R], BF16)     # LayerNormed c (gamma/beta folded out)
        gh0 = big.tile([128, R], BF16)     # gate hidden 1 (= elu+1)
        gh1 = big.tile([128, R], BF16)
        eL = big.tile([8, R], BF16)        # exp(gate logits)
        coeffN = big.tile([8, R], BF16)    # softmax coeffs
        s0a = big.tile([128, R], BF16)     # layer0 out (= elu+1), feat 0..127
        s0b = big.tile([128, R], BF16)     # feat 128..255
        s1a = big.tile([128, R], BF16)
        s1b = big.tile([128, R], BF16)
        zs = [big.tile([128, R], BF16, name=f"zs{q}") for q in range(2)]
        cball = big.tile([128, E * R], BF16)   # per-expert coeff broadcast
        cbz = [big.tile([128, R], BF16, name=f"cbz{q}") for q in range(2)]
        cbe16 = big.tile([128, R], BF16)
        cb = [cball[:, e * R:(e + 1) * R] for e in range(E)]
        otb = big.tile([128, NCH * ACTD], F32)

        # ---------------- stage A: LayerNorm stats ----------------
        mv8 = sp.tile([128, 16], F32, tag="mv8", bufs=1)
        for j in range(NCH):
            ct = ctall[:, 128 * j:128 * (j + 1)]
            stats = sp.tile([128, 6], F32, tag="st")
            nc.vector.bn_stats(stats[:], ct[:])
            nc.vector.bn_aggr(mv8[:, 2 * j:2 * j + 2], stats[:])
        var8 = AP(mv8[:].tensor, mv8[:].offset + 1, [list(mv8[:].ap[0]), [2, NCH]])
        lnv8 = sp.tile([128, NCH], F32, tag="sd", bufs=1)
        nc.scalar.activation(lnv8[:], var8, AF.Ln, bias=epsc[:])
        rstd8 = sp.tile([128, NCH], F32, tag="rs", bufs=1)
        nc.scalar.activation(rstd8[:], lnv8[:], AF.Exp, scale=-0.5)

        def ln_chunk(j):
            js = slice(128 * j, 128 * (j + 1))
            ct = ctall[:, js]
            y = sc.tile([128, 128], BF16, tag="y")
            nc.vector.tensor_scalar(y[:], ct[:], mv8[:, 2 * j:2 * j + 1],
                                    rstd8[:, j:j + 1], OP.subtract, OP.mult)
            yT = pm.tile([128, 128], BF16, tag="mm", name=f"tp{j}")
            nc.tensor.transpose(yT[:], y[:], wsl("i16", 0, 128, 0, 128))
            nc.scalar.copy(cnT[:, js], yT[:])

        # ---------------- stage B: gate + coeff broadcast ----------------
        # DRAM staging: quarter q occupies rows 8q..8q+8 of [32, BQ]
        cstage = dstage.tile([4 * 8, BQ], BF16)
        ctens = cstage.tensor

        def gate_mlp_layer(q, win, bvec, dst):
            qs = slice(BQ * q, BQ * (q + 1))
            pre = pm.tile([128, BQ], F32, tag="mm", name=f"{win}_{q}")
            if win == "g0":
                nc.tensor.matmul(pre[:], wsl("g0z", 0, 32, 0, 128),
                                 zrep[0:32, qs], start=True, stop=False)
                nc.tensor.matmul(pre[:], wsl("g0c", 0, 128, 0, 128),
                                 cnT[:, qs], start=False, stop=True)
            else:
                nc.tensor.matmul(pre[:], wsl("g1w", 0, 128, 0, 128),
                                 gh0[:, qs], start=True, stop=True)
            ee = er.tile([128, BQ], BF16, tag="eg")
            nc.scalar.activation(ee[:], pre[:], AF.Exp, bias=bvec[:])
            rr = er.tile([128, BQ], BF16, tag="rg")
            nc.vector.tensor_scalar(rr[:], pre[:], bvec[:], 0.0, OP.add, OP.max)
            nc.vector.scalar_tensor_tensor(dst[:, qs], ee[:], 1.0, rr[:],
                                           OP.min, OP.add)

        def gate_l2_quarter(q):
            qs = slice(BQ * q, BQ * (q + 1))
            pre2 = psm.tile([8, BQ], F32, tag="sm")
            nc.tensor.matmul(pre2[:], wsl("g2w", 0, 128, 0, 8), gh1[:, qs],
                             start=True, stop=True)
            nc.scalar.activation(eL[:, qs], pre2[:], AF.Exp, bias=g2b[:])
            sume8 = psm.tile([8, BQ], F32, tag="sm")
            nc.tensor.matmul(sume8[:], wsl("on88", 0, 8, 0, 8), eL[:, qs],
                             start=True, stop=True)
            rsum8 = sp.tile([8, BQ], F32, tag="rsm")
            nc.vector.reciprocal_approx_fast(rsum8[:], sume8[:])
            nc.vector.tensor_mul(coeffN[:, qs], eL[:, qs], rsum8[:])
            # stage quarter to a contiguous DRAM block, broadcast back with
            # 4KB-contiguous-source descriptors
            eng_main = nc.sync if q % 2 == 0 else nc.scalar
            eng_aux = nc.scalar if q % 2 == 0 else nc.sync
            qo = 8 * BQ * q
            eng_main.dma_start(AP(ctens, qo, [[BQ, 8], [1, BQ]]), coeffN[:, qs])
            eng_main.dma_start(
                AP(cball.tensor, BQ * q, [[E * R, 128], [R, E], [1, BQ]]),
                AP(ctens, qo, [[0, 128], [1, E * BQ]]))
            if q >= 2:
                for qz in range(2):
                    eng_aux.dma_start(
                        cbz[qz][:, qs],
                        AP(ctens, qo + 4 * BQ * qz, [[BQ, 4], [0, 32], [1, BQ]]))
            eng_aux.dma_start(
                cbe16[:, qs],
                AP(ctens, qo, [[BQ, 8], [0, 16], [1, BQ]]))
            if q == 3:
                bs = slice(BG, 2 * BG)
                for qz in range(2):
                    nc.vector.tensor_mul(zs[qz][:, bs], zrep[:, bs],
                                         cbz[qz][:, bs])

        # ---------------- expert layers (per half) ----------------
        def elu_plus1(ps, dst, bs, tagsfx, fine=False):
            ee = er.tile([128, BG], BF16, tag="e" + tagsfx)
            nc.scalar.activation(ee[:], ps[:], AF.Exp)
            rr = er.tile([128, BG], BF16, tag="r" + tagsfx)
            nc.scalar.activation(rr[:], ps[:], AF.Relu)
            if fine:
                for u in range(2):
                    us = slice(BQ * u, BQ * (u + 1))
                    ds = slice(bs.start + BQ * u, bs.start + BQ * (u + 1))
                    nc.vector.scalar_tensor_tensor(dst[:, ds], ee[:, us], 1.0,
                                                   rr[:, us], OP.min, OP.add)
            else:
                nc.vector.scalar_tensor_tensor(dst[:, bs], ee[:], 1.0, rr[:],
                                               OP.min, OP.add)

        def expert_layer(h, wzn, whn, bias_off, srcs, tag, dsts,
                         pe_bcast=False, fine_tail=False):
            bs = slice(BG * h, BG * (h + 1))
            nkt = len(srcs)
            ps = [pm.tile([128, BG], F32, tag="mm", name=f"ps{tag}{h}_{mt}")
                  for mt in range(2)]
            for mt in range(2):
                nc.tensor.matmul(ps[mt][:],
                                 wsl("b01", 0, 8, bias_off + 128 * mt, 128),
                                 coeffN[:, bs], start=True, stop=False)
            if pe_bcast:
                # bootstrap h0: broadcast coeffs on the PE (one-hot matmul
                # into PSUM) instead of waiting for the DMA round-trip; also
                # build zs from a PE-broadcast selector.
                for qz in range(2):
                    czP = pm.tile([128, BG], F32, tag="cbp", bufs=2,
                                  name=f"czp{qz}")
                    nc.tensor.matmul(czP[:], wsl("sel8", 0, 8, 128 * qz, 128),
                                     coeffN[:, bs], start=True, stop=True)
                    nc.vector.tensor_mul(zs[qz][:, bs], zrep[:, bs], czP[:])
            for kt in range(nkt):
                e, srct = srcs[kt]
                t = sc.tile([128, BG], BF16, tag=tag, name=f"x{tag}{h}_{kt}")
                if pe_bcast:
                    cbP = pm.tile([128, BG], F32, tag="cbp", bufs=2,
                                  name=f"cbp{kt}")
                    nc.tensor.matmul(cbP[:], wsl("ohe", 0, 8, 128 * e, 128),
                                     coeffN[:, bs], start=True, stop=True)
                    nc.vector.tensor_mul(t[:], srct[:, bs], cbP[:])
                else:
                    nc.vector.tensor_mul(t[:], srct[:, bs], cb[e][:, bs])
                for mt in range(2):
                    nc.tensor.matmul(ps[mt][:],
                                     wsl(whn, 0, 128, 256 * kt + 128 * mt, 128),
                                     t[:, :], start=False, stop=False)
            for kt in range(2):
                for mt in range(2):
                    nc.tensor.matmul(ps[mt][:],
                                     wsl(wzn, 0, 128, 256 * kt + 128 * mt, 128),
                                     zs[kt][:, bs], start=False,
                                     stop=(kt == 1 and mt == 1))
            for mt in range(2):
                elu_plus1(ps[mt], dsts[mt], bs, tag, fine=fine_tail)

        def layer2(h, fine=False):
            bs = slice(BG * h, BG * (h + 1))
            per2 = pm.tile([128, BG], F32, tag="mm", name=f"l2_{h}")
            nu = 2 if fine else 1
            w = BG // nu
            for u in range(nu):
                us = slice(w * u, w * (u + 1))
                ds = slice(bs.start + w * u, bs.start + w * (u + 1))
                nc.tensor.matmul(per2[:, us], wsl("w2s", 0, 32, 0, 128),
                                 zrep[0:32, ds], start=True, stop=False)
                nc.tensor.matmul(per2[:, us], wsl("w2s", 0, 128, 128, 128),
                                 s1a[:, ds], start=False, stop=False)
                nc.tensor.matmul(per2[:, us], wsl("w2s", 0, 128, 256, 128),
                                 s1b[:, ds], start=False, stop=True)
                mixed = er.tile([128, w], BF16, tag="mx")
                nc.vector.scalar_tensor_tensor(mixed[:], per2[:, us], b2c[:],
                                               cbe16[:, ds], OP.add, OP.mult)
                for jj in range(w // 128):
                    r = (BG // 128) * h + (w // 128) * u + jj
                    op = pm.tile([128, ACTD], F32, tag="cbp", bufs=2)
                    nc.tensor.matmul(op[:], mixed[:, 128 * jj:128 * (jj + 1)],
                                     wsl("s2", 0, 128, 0, 16), start=True,
                                     stop=True)
                    nc.scalar.copy(otb[:, ACTD * r:ACTD * (r + 1)], op[:])
            # ship this half: partition p holds out rows 8p+4h..8p+4h+4
            if fine:
                nc.sync.dma_start(
                    AP(out_d.tensor, 64 * h, [[NCH * ACTD, 128], [1, 32]]),
                    otb[:, 64 * h:64 * h + 32])
                nc.sync.dma_start(
                    AP(out_d.tensor, 64 * h + 32, [[NCH * ACTD, 128], [1, 32]]),
                    otb[:, 64 * h + 32:64 * h + 64])
            else:
                nc.sync.dma_start(
                    AP(out_d.tensor, 64 * h, [[NCH * ACTD, 128], [1, 64]]),
                    otb[:, 64 * h:64 * h + 64])

        # ---------------- emission order (gate pipelined by quarter) --------
        ln_chunk(0)
        ln_chunk(1)
        gate_mlp_layer(0, "g0", g0b, gh0)
        ln_chunk(2)
        ln_chunk(3)
        gate_mlp_layer(1, "g0", g0b, gh0)
        gate_mlp_layer(0, "g1", g1b, gh1)
        ln_chunk(4)
        ln_chunk(5)
        gate_mlp_layer(2, "g0", g0b, gh0)
        gate_mlp_layer(1, "g1", g1b, gh1)
        gate_l2_quarter(0)
        ln_chunk(6)
        ln_chunk(7)
        gate_mlp_layer(3, "g0", g0b, gh0)
        gate_mlp_layer(2, "g1", g1b, gh1)
        gate_l2_quarter(1)
        gate_mlp_layer(3, "g1", g1b, gh1)
        gate_l2_quarter(2)
        gate_l2_quarter(3)
        expert_layer(0, "w0z", "w0c", 0,
                     [(e, cnT) for e in range(E)], "sc0", (s0a, s0b),
                     pe_bcast=True)
        expert_layer(1, "w0z", "w0c", 0,
                     [(e, cnT) for e in range(E)], "sc0", (s0a, s0b))
        expert_layer(0, "w1z", "w1h", 256,
                     [(e, t) for e in range(E) for t in (s0a, s0b)],
                     "sc1", (s1a, s1b))
        layer2(0)
        expert_layer(1, "w1z", "w1h", 256,
                     [(e, t) for e in range(E) for t in (s0a, s0b)],
                     "sc1", (s1a, s1b), fine_tail=True)
        layer2(1, fine=True)

    nc.compile()
    return nc


def _host_prep(inputs):
    f = lambda a: np.ascontiguousarray(np.asarray(a, dtype=np.float32))
    w0, b0 = f(inputs["w0"]), f(inputs["b0"])
    w1, b1 = f(inputs["w1"]), f(inputs["b1"])
    w2, b2 = f(inputs["w2"]), f(inputs["b2"])
    g0w, g0b = f(inputs["g0w"]), f(inputs["g0b"])
    g1w, g1b = f(inputs["g1w"]), f(inputs["g1b"])
    g2w, g2b = f(inputs["g2w"]), f(inputs["g2b"])
    ln_g, ln_b = f(inputs["ln_g"]), f(inputs["ln_b"])

    # fold LayerNorm gamma/beta into the c-consuming weights and biases:
    # on-chip cn = (c - mu) * rstd; true cn = cn_core*gamma + beta.
    w0c = w0[:, LATENT:, :]                       # [E, CIN, HID]
    w0c_g = w0c * ln_g[None, :, None]
    b0_f = b0 + np.einsum("f,efo->eo", ln_b, w0c)
    g0c = g0w[LATENT:]                            # [CIN, GH]
    g0c_g = g0c * ln_g[:, None]
    g0b_f = g0b + ln_b @ g0c

    def ksb(wstk, nkt, m):   # [nkt*128, m] -> [128, nkt*m]
        return np.ascontiguousarray(
            wstk.reshape(nkt, 128, m).transpose(1, 0, 2).reshape(128, nkt * m))

    wall = np.zeros((128, NWALL), np.float32)
    wgate = np.zeros((128, NGATE), np.float32)
    def put(name, arr):
        if name in _GOFF:
            o = _GOFF[name]
            wgate[:arr.shape[0], o:o + arr.shape[1]] = arr
        else:
            o = _WOFF[name]
            wall[:arr.shape[0], o:o + arr.shape[1]] = arr

    put("w0z", ksb(w0[:, :LATENT, :].reshape(E * LATENT, HID), 2, HID))
    put("w0c", ksb(w0c_g.reshape(E * CIN, HID), 8, HID))
    put("w1z", ksb(w1[:, :LATENT, :].reshape(E * LATENT, HID), 2, HID))
    put("w1h", ksb(w1[:, LATENT:, :].reshape(E * HID, HID), 16, HID))
    w2stk = w2.transpose(1, 0, 2).reshape(INTER, E * ACTD)   # [288, 128]
    w2s = np.zeros((128, 384), np.float32)
    w2s[:32, 0:128] = w2stk[0:32]
    w2s[:, 128:256] = w2stk[32:160]
    w2s[:, 256:384] = w2stk[160:288]
    put("w2s", w2s)
    put("s2", np.tile(np.eye(ACTD, dtype=np.float32), (E, 1)))
    put("g0z", g0w[:LATENT])
    put("g0c", g0c_g)
    put("g1w", g1w)
    put("g2w", g2w)
    b1f = b1 - w1[:, LATENT:, :].sum(axis=1)
    put("b01", np.concatenate([b0_f, b1f], axis=1))
    put("on88", np.ones((8, 8), np.float32))
    put("i16", np.eye(128, dtype=np.float32))
    ohe = np.zeros((8, 8 * 128), np.float32)
    for e in range(8):
        ohe[e, 128 * e:128 * (e + 1)] = 1.0
    put("ohe", ohe)
    sel8 = np.zeros((8, 256), np.float32)
    for qz in range(2):
        for j in range(4):
            sel8[4 * qz + j, 128 * qz + 32 * j:128 * qz + 32 * (j + 1)] = 1.0
    put("sel8", sel8)

    b2f = b2 - w2[:, LATENT:, :].sum(axis=1)                 # [8,16]
    consts = np.zeros((128, 6), np.float32)
    consts[:, 0] = LN_EPS
    consts[:, 1] = g0b_f
    consts[:, 2] = g1b - g1w.sum(0)
    consts[:, 3] = b2f.reshape(128)
    consts[:8, 4] = (g2b - g2w.sum(0))
    return {"wall": wall.astype(NP_BF16), "wgate": wgate.astype(NP_BF16),
            "consts": consts}


def make_in_maps(inputs):
    wmap = _host_prep(inputs)
    z = np.ascontiguousarray(np.asarray(inputs["z"], dtype=np.float32))
    c = np.ascontiguousarray(np.asarray(inputs["c"], dtype=np.float32))
    # on-chip batch order: i = 128*r + p  <->  original row b = 8p + r
    ii = np.arange(R)
    perm = 8 * (ii % 128) + ii // 128
    in_maps = []
    for i in range(N_CORES):
        m = dict(wmap)
        zsh = z[i * R:(i + 1) * R]
        m["zrep"] = np.ascontiguousarray(np.tile(zsh.T[:, perm], (4, 1))).astype(NP_BF16)
        csh = c[i * R:(i + 1) * R]
        # partition p <- rows 8p..8p+8 (contiguous 4KB lines)
        m["cperm"] = np.ascontiguousarray(csh.reshape(128, NCH * CIN)).astype(NP_BF16)
        in_maps.append(m)
    return in_maps


def kernel(**inputs):
    if "nc" not in _CACHE:
        _CACHE["nc"] = _build_program()
    nc = _CACHE["nc"]
    in_maps = make_in_maps(inputs)
    res = bass_utils.run_bass_kernel_spmd(nc, in_maps, core_ids=list(range(N_CORES)))
    return np.concatenate([res.results[i]["out"] for i in range(N_CORES)], axis=0)


# revision 22
# speedup vs baseline: 1.0343x; 1.0343x over previous
"""Trainium2 Bass kernel for nn_MixedMlp (soft-mixture MoE MLP).

Math (per batch row b):
    cn = LayerNorm(c); x = [z, cn]
    coeff = softmax(gateMLP(x))                       # [E]
    l0 = elu(sum_e coeff_e (x @ w0_e + b0_e))
    l1 = elu(sum_e coeff_e ([z, l0] @ w1_e + b1_e))
    out = sum_e coeff_e ([z, l1] @ w2_e + b2_e)

Kernel strategy (8 cores, data-parallel over B=8192):
  * Activations feature-major ([features, batch]); every layer is one
    PSUM-accumulated GEMM contracting coeff-scaled inputs:
    out^T = sum_e W_e^T (coeff_e . X^T).  bfloat16 on-chip.
  * LayerNorm gamma/beta folded into W0c/g0c and biases host-side; LN
    stats computed on the Scalar engine via activation accum_out
    (sum x, sum x^2), freeing the DVE.
  * ELU as s = elu(x)+1 = relu(x) + min(exp(x), 1), -1 folded into next
    bias; exp on Scalar, relu on Vector (gate) or Scalar (expert phase),
    combine on DVE.
  * Softmax sum-of-exps lands on 8 partitions via a ones[8,8] matmul ->
    8-lane reciprocal -> one multiply.
  * Gate output processed in four 256-col quarters: each quarter's coeffs
    are staged to a contiguous DRAM block and broadcast back with
    4KB-contiguous-source stride-0 DMAs (fast descriptors), so expert
    layers start as soon as the first quarter's broadcast lands.
  * Expert layers per 512-col half; scaled-input products split between
    DVE and GpSimd; h1's softmax/broadcast chain is emitted between L0-h0's
    bias and product matmuls to keep the PE busy.
  * Activation-function table pinned once (natural_log_exp set) at t=0.
"""

import numpy as np
import ml_dtypes
from contextlib import ExitStack

import concourse.bass as bass
import concourse.bacc as bacc
import concourse.tile as tile
import concourse.mybir as mybir
from concourse import bass_utils
from concourse.bass import AP

F32 = mybir.dt.float32
BF16 = mybir.dt.bfloat16
AF = mybir.ActivationFunctionType
OP = mybir.AluOpType
NP_BF16 = ml_dtypes.bfloat16

N_CORES = 8
B = 8192
R = B // N_CORES          # rows per core = 1024
LATENT, CIN, HID, ACTD, E, GH = 32, 128, 256, 16, 8, 128
IN0, INTER = LATENT + CIN, HID + LATENT
LN_EPS = 1e-5
BG = 512                  # half width
BQ = 256                  # quarter width
NCH = R // 128            # 8 b-chunks per core

_GCOLS = [("g0z", 128), ("g0c", 128), ("g1w", 128), ("g2w", 8),
          ("b01", 512), ("on88", 8), ("i16", 128), ("ohe", 1024),
          ("sel8", 256)]
_WCOLS = [("w0z", 512), ("w0c", 2048), ("w1z", 512), ("w1h", 4096),
          ("w2s", 384), ("s2", 16)]
_GOFF, _WOFF = {}, {}
_o = 0
for _n, _c in _GCOLS:
    _GOFF[_n] = _o
    _o += _c
NGATE = _o
_o = 0
for _n, _c in _WCOLS:
    _WOFF[_n] = _o
    _o += _c
NWALL = _o
WSPLIT = _WOFF["w1z"]     # wall_a = w0 weights, wall_b = w1/w2 weights

ACT_SET_LN_EXP = 6        # natural_log_exp_and_others in act_info.json
N_GP0 = 3                 # scaled inputs on gpsimd per half, layer 0
N_GP1 = 5                 # and layer 1

_CACHE = {}


def _build_program():
    nc = bacc.Bacc("TRN2", target_bir_lowering=False, debug=False,
                   num_devices=N_CORES)

    zr_d = nc.dram_tensor("zrep", [128, R], BF16, kind="ExternalInput").ap()
    c_d = nc.dram_tensor("cperm", [128, NCH * CIN], BF16, kind="ExternalInput").ap()
    wg_d = nc.dram_tensor("wgate", [128, NGATE], BF16, kind="ExternalInput").ap()
    wall_d = nc.dram_tensor("wall", [128, NWALL], BF16, kind="ExternalInput").ap()
    ck_d = nc.dram_tensor("consts", [128, 6], F32, kind="ExternalInput").ap()
    out_d = nc.dram_tensor("out", [R, ACTD], F32, kind="ExternalOutput").ap()

    with tile.TileContext(nc) as tc, ExitStack() as ctx:
        wp = ctx.enter_context(tc.tile_pool(name="wp", bufs=1))       # weights
        big = ctx.enter_context(tc.tile_pool(name="big", bufs=1))     # persistent activations
        sp = ctx.enter_context(tc.tile_pool(name="sp", bufs=4))       # small temps
        er = ctx.enter_context(tc.tile_pool(name="er", bufs=6))       # elu temps
        sc = ctx.enter_context(tc.tile_pool(name="sc", bufs=6))       # scaled-input tiles
        pm = ctx.enter_context(tc.tile_pool(name="pm", bufs=4, space="PSUM"))   # big matmuls
        psm = ctx.enter_context(tc.tile_pool(name="psm", bufs=1, space="PSUM")) # small matmuls
        dstage = ctx.enter_context(tc.tile_pool(name="dstage", bufs=1, space="DRAM"))

        # pin the activation table (ln+exp+relu+copy+square) once, at t=0
        nc.scalar.add_instruction(mybir.InstLoadActFuncSet(
            name=nc.get_next_instruction_name(),
            act_func_set_id=ACT_SET_LN_EXP, ins=[], outs=[]))

        # ---------------- bulk loads (priority-ordered per HWDGE queue) -----
        # c pre-permuted: partition p holds rows 8p..8p+8.  on-chip batch
        # order is i = 128*r + p  <->  original row b = 8p + r.
        ctall = big.tile([128, NCH * CIN], BF16)
        for jj in range(4):
            nc.sync.dma_start(ctall[:, 2 * CIN * jj:2 * CIN * (jj + 1)],
                              c_d[:, 2 * CIN * jj:2 * CIN * (jj + 1)])
        wall = wp.tile([128, NWALL], BF16)
        nc.sync.dma_start(wall[:, 0:WSPLIT], wall_d[:, 0:WSPLIT],
                          max_dma_last_dim=4096)
        nc.sync.dma_start(wall[:, WSPLIT:], wall_d[:, WSPLIT:],
                          max_dma_last_dim=4096)
        wgate = wp.tile([128, NGATE], BF16)
        nc.scalar.dma_start(wgate[:], wg_d[:], max_dma_last_dim=4096)
        zrep = big.tile([128, R], BF16)
        nc.scalar.dma_start(zrep[:], zr_d[:])
        ckt = wp.tile([128, 6], F32)
        nc.scalar.dma_start(ckt[:], ck_d[:])

        def wsl(name, p0, pn, c0, cn_):
            if name in _GOFF:
                o = _GOFF[name]
                return wgate[p0:p0 + pn, o + c0:o + c0 + cn_]
            o = _WOFF[name]
            return wall[p0:p0 + pn, o + c0:o + c0 + cn_]
        epsc = ckt[:, 0:1]
        g0b, g1b, b2c = ckt[:, 1:2], ckt[:, 2:3], ckt[:, 3:4]
        g2b = ckt[0:8, 4:5]

        # ---------------- persistent activation tiles ----------------
        cnT = big.tile([128, R], BF16)     # LayerNormed c (gamma/beta folded out)
        gh0 = big.tile([128, R], BF16)     # gate hidden 1 (= elu+1)
        gh1 = big.tile([128, R], BF16)
        eL = big.tile([8, R], BF16)        # exp(gate logits)
        coeffN = big.tile([8, R], BF16)    # softmax coeffs
        s0a = big.tile([128, R], BF16)     # layer0 out (= elu+1), feat 0..127
        s0b = big.tile([128, R], BF16)     # feat 128..255
        s1a = big.tile([128, R], BF16)
        s1b = big.tile([128, R], BF16)
        zs = [big.tile([128, R], BF16, name=f"zs{q}") for q in range(2)]
        cball = big.tile([128, E * R], BF16)   # per-expert coeff broadcast
        cbz = [big.tile([128, R], BF16, name=f"cbz{q}") for q in range(2)]
        cbe16 = big.tile([128, R], BF16)
        cb = [cball[:, e * R:(e + 1) * R] for e in range(E)]
        otb = big.tile([128, NCH * ACTD], F32)

        # ---------------- stage A: LayerNorm stats ----------------
        mv8 = sp.tile([128, 16], F32, tag="mv8", bufs=1)
        for j in range(NCH):
            ct = ctall[:, 128 * j:128 * (j + 1)]
            stats = sp.tile([128, 6], F32, tag="st")
            nc.vector.bn_stats(stats[:], ct[:])
            nc.vector.bn_aggr(mv8[:, 2 * j:2 * j + 2], stats[:])
        var8 = AP(mv8[:].tensor, mv8[:].offset + 1, [list(mv8[:].ap[0]), [2, NCH]])
        lnv8 = sp.tile([128, NCH], F32, tag="sd", bufs=1)
        nc.scalar.activation(lnv8[:], var8, AF.Ln, bias=epsc[:])
        rstd8 = sp.tile([128, NCH], F32, tag="rs", bufs=1)
        nc.scalar.activation(rstd8[:], lnv8[:], AF.Exp, scale=-0.5)

        def ln_chunk(j):
            js = slice(128 * j, 128 * (j + 1))
            ct = ctall[:, js]
            y = sc.tile([128, 128], BF16, tag="y")
            nc.vector.tensor_scalar(y[:], ct[:], mv8[:, 2 * j:2 * j + 1],
                                    rstd8[:, j:j + 1], OP.subtract, OP.mult)
            yT = pm.tile([128, 128], BF16, tag="mm", name=f"tp{j}")
            nc.tensor.transpose(yT[:], y[:], wsl("i16", 0, 128, 0, 128))
            nc.scalar.copy(cnT[:, js], yT[:])

        # ---------------- stage B: gate + coeff broadcast ----------------
        # DRAM staging: quarter q occupies rows 8q..8q+8 of [32, BQ]
        cstage = dstage.tile([4 * 8, BQ], BF16)
        ctens = cstage.tensor

        def gate_mlp_layer(q, win, bvec, dst):
            qs = slice(BQ * q, BQ * (q + 1))
            pre = pm.tile([128, BQ], F32, tag="mm", name=f"{win}_{q}")
            if win == "g0":
                nc.tensor.matmul(pre[:], wsl("g0z", 0, 32, 0, 128),
                                 zrep[0:32, qs], start=True, stop=False)
                nc.tensor.matmul(pre[:], wsl("g0c", 0, 128, 0, 128),
                                 cnT[:, qs], start=False, stop=True)
            else:
                nc.tensor.matmul(pre[:], wsl("g1w", 0, 128, 0, 128),
                                 gh0[:, qs], start=True, stop=True)
            ee = er.tile([128, BQ], BF16, tag="eg")
            nc.scalar.activation(ee[:], pre[:], AF.Exp, bias=bvec[:])
            rr = er.tile([128, BQ], BF16, tag="rg")
            nc.vector.tensor_scalar(rr[:], pre[:], bvec[:], 0.0, OP.add, OP.max)
            nc.vector.scalar_tensor_tensor(dst[:, qs], ee[:], 1.0, rr[:],
                                           OP.min, OP.add)

        def gate_l2_quarter(q):
            qs = slice(BQ * q, BQ * (q + 1))
            pre2 = psm.tile([8, BQ], F32, tag="sm")
            nc.tensor.matmul(pre2[:], wsl("g2w", 0, 128, 0, 8), gh1[:, qs],
                             start=True, stop=True)
            nc.scalar.activation(eL[:, qs], pre2[:], AF.Exp, bias=g2b[:])
            sume8 = psm.tile([8, BQ], F32, tag="sm")
            nc.tensor.matmul(sume8[:], wsl("on88", 0, 8, 0, 8), eL[:, qs],
                             start=True, stop=True)
            rsum8 = sp.tile([8, BQ], F32, tag="rsm")
            nc.vector.reciprocal_approx_fast(rsum8[:], sume8[:])
            nc.vector.tensor_mul(coeffN[:, qs], eL[:, qs], rsum8[:])
            # stage quarter to a contiguous DRAM block, broadcast back with
            # 4KB-contiguous-source descriptors
            eng_main = nc.sync if q % 2 == 0 else nc.scalar
            eng_aux = nc.scalar if q % 2 == 0 else nc.sync
            qo = 8 * BQ * q
            eng_main.dma_start(AP(ctens, qo, [[BQ, 8], [1, BQ]]), coeffN[:, qs])
            eng_main.dma_start(
                AP(cball.tensor, BQ * q, [[E * R, 128], [R, E], [1, BQ]]),
                AP(ctens, qo, [[0, 128], [1, E * BQ]]))
            if q >= 2:
                for qz in range(2):
                    eng_aux.dma_start(
                        cbz[qz][:, qs],
                        AP(ctens, qo + 4 * BQ * qz, [[BQ, 4], [0, 32], [1, BQ]]))
            eng_aux.dma_start(
                cbe16[:, qs],
                AP(ctens, qo, [[BQ, 8], [0, 16], [1, BQ]]))
            if q == 3:
                bs = slice(BG, 2 * BG)
                for qz in range(2):
                    nc.vector.tensor_mul(zs[qz][:, bs], zrep[:, bs],
                                         cbz[qz][:, bs])

        # ---------------- expert layers (per half) ----------------
        def elu_plus1(ps, dst, bs, tagsfx, fine=False):
            ee = er.tile([128, BG], BF16, tag="e" + tagsfx)
            nc.scalar.activation(ee[:], ps[:], AF.Exp)
            rr = er.tile([128, BG], BF16, tag="r" + tagsfx)
            nc.scalar.activation(rr[:], ps[:], AF.Relu)
            if fine:
                for u in range(2):
                    us = slice(BQ * u, BQ * (u + 1))
                    ds = slice(bs.start + BQ * u, bs.start + BQ * (u + 1))
                    nc.vector.scalar_tensor_tensor(dst[:, ds], ee[:, us], 1.0,
                                                   rr[:, us], OP.min, OP.add)
            else:
                nc.vector.scalar_tensor_tensor(dst[:, bs], ee[:], 1.0, rr[:],
                                               OP.min, OP.add)

        def expert_layer(h, wzn, whn, bias_off, srcs, tag, dsts,
                         pe_bcast=False, fine_tail=False):
            bs = slice(BG * h, BG * (h + 1))
            nkt = len(srcs)
            ps = [pm.tile([128, BG], F32, tag="mm", name=f"ps{tag}{h}_{mt}")
                  for mt in range(2)]
            for mt in range(2):
                nc.tensor.matmul(ps[mt][:],
                                 wsl("b01", 0, 8, bias_off + 128 * mt, 128),
                                 coeffN[:, bs], start=True, stop=False)
            if pe_bcast:
                # bootstrap h0: broadcast coeffs on the PE (one-hot matmul
                # into PSUM) instead of waiting for the DMA round-trip; also
                # build zs from a PE-broadcast selector.
                for qz in range(2):
                    czP = pm.tile([128, BG], F32, tag="cbp", bufs=2,
                                  name=f"czp{qz}")
                    nc.tensor.matmul(czP[:], wsl("sel8", 0, 8, 128 * qz, 128),
                                     coeffN[:, bs], start=True, stop=True)
                    nc.vector.tensor_mul(zs[qz][:, bs], zrep[:, bs], czP[:])
            for kt in range(nkt):
                e, srct = srcs[kt]
                t = sc.tile([128, BG], BF16, tag=tag, name=f"x{tag}{h}_{kt}")
                if pe_bcast:
                    cbP = pm.tile([128, BG], F32, tag="cbp", bufs=2,
                                  name=f"cbp{kt}")
                    nc.tensor.matmul(cbP[:], wsl("ohe", 0, 8, 128 * e, 128),
                                     coeffN[:, bs], start=True, stop=True)
                    nc.vector.tensor_mul(t[:], srct[:, bs], cbP[:])
                else:
                    nc.vector.tensor_mul(t[:], srct[:, bs], cb[e][:, bs])
                for mt in range(2):
                    nc.tensor.matmul(ps[mt][:],
                                     wsl(whn, 0, 128, 256 * kt + 128 * mt, 128),
                                     t[:, :], start=False, stop=False)
            for kt in range(2):
                for mt in range(2):
                    nc.tensor.matmul(ps[mt][:],
                                     wsl(wzn, 0, 128, 256 * kt + 128 * mt, 128),
                                     zs[kt][:, bs], start=False,
                                     stop=(kt == 1 and mt == 1))
            for mt in range(2):
                elu_plus1(ps[mt], dsts[mt], bs, tag, fine=fine_tail)

        def layer2(h, fine=False):
            bs = slice(BG * h, BG * (h + 1))
            per2 = pm.tile([128, BG], F32, tag="mm", name=f"l2_{h}")
            nu = 2 if fine else 1
            w = BG // nu
            for u in range(nu):
                us = slice(w * u, w * (u + 1))
                ds = slice(bs.start + w * u, bs.start + w * (u + 1))
                nc.tensor.matmul(per2[:, us], wsl("w2s", 0, 32, 0, 128),
                                 zrep[0:32, ds], start=True, stop=False)
                nc.tensor.matmul(per2[:, us], wsl("w2s", 0, 128, 128, 128),
                                 s1a[:, ds], start=False, stop=False)
                nc.tensor.matmul(per2[:, us], wsl("w2s", 0, 128, 256, 128),
                                 s1b[:, ds], start=False, stop=True)
                mixed = er.tile([128, w], BF16, tag="mx")
                nc.vector.scalar_tensor_tensor(mixed[:], per2[:, us], b2c[:],
                                               cbe16[:, ds], OP.add, OP.mult)
                for jj in range(w // 128):
                    r = (BG // 128) * h + (w // 128) * u + jj
                    op = pm.tile([128, ACTD], F32, tag="cbp", bufs=2)
                    nc.tensor.matmul(op[:], mixed[:, 128 * jj:128 * (jj + 1)],
                                     wsl("s2", 0, 128, 0, 16), start=True,
                                     stop=True)
                    nc.scalar.copy(otb[:, ACTD * r:ACTD * (r + 1)], op[:])
            # ship this half: partition p holds out rows 8p+4h..8p+4h+4
            if fine:
                nc.sync.dma_start(
                    AP(out_d.tensor, 64 * h, [[NCH * ACTD, 128], [1, 32]]),
                    otb[:, 64 * h:64 * h + 32])
                nc.sync.dma_start(
                    AP(out_d.tensor, 64 * h + 32, [[NCH * ACTD, 128], [1, 32]]),
                    otb[:, 64 * h + 32:64 * h + 64])
            else:
                nc.sync.dma_start(
                    AP(out_d.tensor, 64 * h, [[NCH * ACTD, 128], [1, 64]]),
                    otb[:, 64 * h:64 * h + 64])

        # ---------------- emission order (gate pipelined by quarter) --------
        ln_chunk(0)
        ln_chunk(1)
        gate_mlp_layer(0, "g0", g0b, gh0)
        ln_chunk(2)
        ln_chunk(3)
        gate_mlp_layer(1, "g0", g0b, gh0)
        gate_mlp_layer(0, "g1", g1b, gh1)
        ln_chunk(4)
        ln_chunk(5)
        gate_mlp_layer(2, "g0", g0b, gh0)
        gate_mlp_layer(1, "g1", g1b, gh1)
        gate_l2_quarter(0)
        ln_chunk(6)
        ln_chunk(7)
        gate_mlp_layer(3, "g0", g0b, gh0)
        gate_mlp_layer(2, "g1", g1b, gh1)
        gate_l2_quarter(1)
        gate_mlp_layer(3, "g1", g1b, gh1)
        gate_l2_quarter(2)
        gate_l2_quarter(3)
        expert_layer(0, "w0z", "w0c", 0,
                     [(e, cnT) for e in range(E)], "sc0", (s0a, s0b),
                     pe_bcast=True)
        expert_layer(1, "w0z", "w0c", 0,
                     [(e, cnT) for e in range(E)], "sc0", (s0a, s0b))
        expert_layer(0, "w1z", "w1h", 256,
                     [(e, t) for e in range(E) for t in (s0a, s0b)],
                     "sc1", (s1a, s1b))
        layer2(0)
        expert_layer(1, "w1z", "w1h", 256,
                     [(e, t) for e in range(E) for t in (s0a, s0b)],
                     "sc1", (s1a, s1b), fine_tail=True)
        layer2(1, fine=True)

    nc.compile()
    return nc


def _host_prep(inputs):
    f = lambda a: np.ascontiguousarray(np.asarray(a, dtype=np.float32))
    w0, b0 = f(inputs["w0"]), f(inputs["b0"])
    w1, b1 = f(inputs["w1"]), f(inputs["b1"])
    w2, b2 = f(inputs["w2"]), f(inputs["b2"])
    g0w, g0b = f(inputs["g0w"]), f(inputs["g0b"])
    g1w, g1b = f(inputs["g1w"]), f(inputs["g1b"])
    g2w, g2b = f(inputs["g2w"]), f(inputs["g2b"])
    ln_g, ln_b = f(inputs["ln_g"]), f(inputs["ln_b"])

    # fold LayerNorm gamma/beta into the c-consuming weights and biases:
    # on-chip cn = (c - mu) * rstd; true cn = cn_core*gamma + beta.
    w0c = w0[:, LATENT:, :]                       # [E, CIN, HID]
    w0c_g = w0c * ln_g[None, :, None]
    b0_f = b0 + np.einsum("f,efo->eo", ln_b, w0c)
    g0c = g0w[LATENT:]                            # [CIN, GH]
    g0c_g = g0c * ln_g[:, None]
    g0b_f = g0b + ln_b @ g0c

    def ksb(wstk, nkt, m):   # [nkt*128, m] -> [128, nkt*m]
        return np.ascontiguousarray(
            wstk.reshape(nkt, 128, m).transpose(1, 0, 2).reshape(128, nkt * m))

    wall = np.zeros((128, NWALL), np.float32)
    wgate = np.zeros((128, NGATE), np.float32)
    def put(name, arr):
        if name in _GOFF:
            o = _GOFF[name]
            wgate[:arr.shape[0], o:o + arr.shape[1]] = arr
        else:
            o = _WOFF[name]
            wall[:arr.shape[0], o:o + arr.shape[1]] = arr

    put("w0z", ksb(w0[:, :LATENT, :].reshape(E * LATENT, HID), 2, HID))
    put("w0c", ksb(w0c_g.reshape(E * CIN, HID), 8, HID))
    put("w1z", ksb(w1[:, :LATENT, :].reshape(E * LATENT, HID), 2, HID))
    put("w1h", ksb(w1[:, LATENT:, :].reshape(E * HID, HID), 16, HID))
    w2stk = w2.transpose(1, 0, 2).reshape(INTER, E * ACTD)   # [288, 128]
    w2s = np.zeros((128, 384), np.float32)
    w2s[:32, 0:128] = w2stk[0:32]
    w2s[:, 128:256] = w2stk[32:160]
    w2s[:, 256:384] = w2stk[160:288]
    put("w2s", w2s)
    put("s2", np.tile(np.eye(ACTD, dtype=np.float32), (E, 1)))
    put("g0z", g0w[:LATENT])
    put("g0c", g0c_g)
    put("g1w", g1w)
    put("g2w", g2w)
    b1f = b1 - w1[:, LATENT:, :].sum(axis=1)
    put("b01", np.concatenate([b0_f, b1f], axis=1))
    put("on88", np.ones((8, 8), np.float32))
    put("i16", np.eye(128, dtype=np.float32))
    ohe = np.zeros((8, 8 * 128), np.float32)
    for e in range(8):
        ohe[e, 128 * e:128 * (e + 1)] = 1.0
    put("ohe", ohe)
    sel8 = np.zeros((8, 256), np.float32)
    for qz in range(2):
        for j in range(4):
            sel8[4 * qz + j, 128 * qz + 32 * j:128 * qz + 32 * (j + 1)] = 1.0
    put("sel8", sel8)

    b2f = b2 - w2[:, LATENT:, :].sum(axis=1)                 # [8,16]
    consts = np.zeros((128, 6), np.float32)
    consts[:, 0] = LN_EPS
    consts[:, 1] = g0b_f
    consts[:, 2] = g1b - g1w.sum(0)
    consts[:, 3] = b2f.reshape(128)
    consts[:8, 4] = (g2b - g2w.sum(0))
    return {"wall": wall.astype(NP_BF16), "wgate": wgate.astype(NP_BF16),
            "consts": consts}


def make_in_maps(inputs):
    wmap = _host_prep(inputs)
    z = np.ascontiguousarray(np.asarray(inputs["z"], dtype=np.float32))
    c = np.ascontiguousarray(np.asarray(inputs["c"], dtype=np.float32))
    # on-chip batch order: i = 128*r + p  <->  original row b = 8p + r
    ii = np.arange(R)
    perm = 8 * (ii % 128) + ii // 128
    in_maps = []
    for i in range(N_CORES):
        m = dict(wmap)
        zsh = z[i * R:(i + 1) * R]
        m["zrep"] = np.ascontiguousarray(np.tile(zsh.T[:, perm], (4, 1))).astype(NP_BF16)
        csh = c[i * R:(i + 1) * R]
        # partition p <- rows 8p..8p+8 (contiguous 4KB lines)
        m["cperm"] = np.ascontiguousarray(csh.reshape(128, NCH * CIN)).astype(NP_BF16)
        in_maps.append(m)
    return in_maps


def kernel(**inputs):
    if "nc" not in _CACHE:
        _CACHE["nc"] = _build_program()
    nc = _CACHE["nc"]
    in_maps = make_in_maps(inputs)
    res = bass_utils.run_bass_kernel_spmd(nc, in_maps, core_ids=list(range(N_CORES)))
    return np.concatenate([res.results[i]["out"] for i in range(N_CORES)], axis=0)


# revision 23
# speedup vs baseline: 1.1675x; 1.1287x over previous
"""Trainium2 Bass kernel for nn_MixedMlp (soft-mixture MoE MLP).

Math (per batch row b):
    cn = LayerNorm(c); x = [z, cn]
    coeff = softmax(gateMLP(x))                       # [E]
    l0 = elu(sum_e coeff_e (x @ w0_e + b0_e))
    l1 = elu(sum_e coeff_e ([z, l0] @ w1_e + b1_e))
    out = sum_e coeff_e ([z, l1] @ w2_e + b2_e)

Kernel strategy (8 cores, data-parallel over B=8192):
  * Activations feature-major ([features, batch]); every layer is one
    PSUM-accumulated GEMM contracting coeff-scaled inputs:
    out^T = sum_e W_e^T (coeff_e . X^T).  bfloat16 on-chip.
  * LayerNorm gamma/beta folded into W0c/g0c and biases host-side; LN
    stats computed on the Scalar engine via activation accum_out
    (sum x, sum x^2), freeing the DVE.
  * ELU as s = elu(x)+1 = relu(x) + min(exp(x), 1), -1 folded into next
    bias; exp on Scalar, relu on Vector (gate) or Scalar (expert phase),
    combine on DVE.
  * Softmax sum-of-exps lands on 8 partitions via a ones[8,8] matmul ->
    8-lane reciprocal -> one multiply.
  * Gate output processed in four 256-col quarters: each quarter's coeffs
    are staged to a contiguous DRAM block and broadcast back with
    4KB-contiguous-source stride-0 DMAs (fast descriptors), so expert
    layers start as soon as the first quarter's broadcast lands.
  * Expert layers per 512-col half; scaled-input products split between
    DVE and GpSimd; h1's softmax/broadcast chain is emitted between L0-h0's
    bias and product matmuls to keep the PE busy.
  * Activation-function table pinned once (natural_log_exp set) at t=0.
"""

import numpy as np
import ml_dtypes
from contextlib import ExitStack

import concourse.bass as bass
import concourse.bacc as bacc
import concourse.tile as tile
import concourse.mybir as mybir
from concourse import bass_utils
from concourse.bass import AP

F32 = mybir.dt.float32
BF16 = mybir.dt.bfloat16
AF = mybir.ActivationFunctionType
OP = mybir.AluOpType
NP_BF16 = ml_dtypes.bfloat16

N_CORES = 8
B = 8192
R = B // N_CORES          # rows per core = 1024
LATENT, CIN, HID, ACTD, E, GH = 32, 128, 256, 16, 8, 128
IN0, INTER = LATENT + CIN, HID + LATENT
LN_EPS = 1e-5
BG = 512                  # half width
BQ = 256                  # quarter width
NCH = R // 128            # 8 b-chunks per core

_GCOLS = [("g0z", 128), ("g0c", 128), ("g1w", 128), ("g2w", 8),
          ("b01", 512), ("on88", 8), ("i16", 128), ("ohe", 1024),
          ("sel8", 256)]
_WCOLS = [("w0z", 512), ("w0c", 2048), ("w1z", 512), ("w1h", 4096),
          ("w2s", 384), ("s2", 16)]
_GOFF, _WOFF = {}, {}
_o = 0
for _n, _c in _GCOLS:
    _GOFF[_n] = _o
    _o += _c
NGATE = _o
_o = 0
for _n, _c in _WCOLS:
    _WOFF[_n] = _o
    _o += _c
NWALL = _o
WSPLIT = _WOFF["w1z"]     # wall_a = w0 weights, wall_b = w1/w2 weights

ACT_SET_LN_EXP = 6        # natural_log_exp_and_others in act_info.json
N_GP0 = 3                 # scaled inputs on gpsimd per half, layer 0
N_GP1 = 5                 # and layer 1

_CACHE = {}


def _build_program():
    nc = bacc.Bacc("TRN2", target_bir_lowering=False, debug=False,
                   num_devices=N_CORES)

    zr_d = nc.dram_tensor("zrep", [128, R], BF16, kind="ExternalInput").ap()
    c_d = nc.dram_tensor("cperm", [128, NCH * CIN], BF16, kind="ExternalInput").ap()
    wg_d = nc.dram_tensor("wgate", [128, NGATE], BF16, kind="ExternalInput").ap()
    wall_d = nc.dram_tensor("wall", [128, NWALL], BF16, kind="ExternalInput").ap()
    ck_d = nc.dram_tensor("consts", [128, 6], F32, kind="ExternalInput").ap()
    out_d = nc.dram_tensor("out", [R, ACTD], F32, kind="ExternalOutput").ap()

    with tile.TileContext(nc) as tc, ExitStack() as ctx:
        wp = ctx.enter_context(tc.tile_pool(name="wp", bufs=1))       # weights
        big = ctx.enter_context(tc.tile_pool(name="big", bufs=1))     # persistent activations
        sp = ctx.enter_context(tc.tile_pool(name="sp", bufs=4))       # small temps
        er = ctx.enter_context(tc.tile_pool(name="er", bufs=6))       # elu temps
        sc = ctx.enter_context(tc.tile_pool(name="sc", bufs=6))       # scaled-input tiles
        pm = ctx.enter_context(tc.tile_pool(name="pm", bufs=4, space="PSUM"))   # big matmuls
        psm = ctx.enter_context(tc.tile_pool(name="psm", bufs=1, space="PSUM")) # small matmuls
        dstage = ctx.enter_context(tc.tile_pool(name="dstage", bufs=1, space="DRAM"))

        # pin the activation table (ln+exp+relu+copy+square) once, at t=0
        nc.scalar.add_instruction(mybir.InstLoadActFuncSet(
            name=nc.get_next_instruction_name(),
            act_func_set_id=ACT_SET_LN_EXP, ins=[], outs=[]))

        # ---------------- bulk loads (priority-ordered per HWDGE queue) -----
        # c pre-permuted: partition p holds rows 8p..8p+8.  on-chip batch
        # order is i = 128*r + p  <->  original row b = 8p + r.
        ctall = big.tile([128, NCH * CIN], BF16)
        for jj in range(2):
            nc.sync.dma_start(ctall[:, 4 * CIN * jj:4 * CIN * (jj + 1)],
                              c_d[:, 4 * CIN * jj:4 * CIN * (jj + 1)])
        wall = wp.tile([128, NWALL], BF16)
        nc.sync.dma_start(wall[:, 0:WSPLIT], wall_d[:, 0:WSPLIT],
                          max_dma_last_dim=4096)
        nc.sync.dma_start(wall[:, WSPLIT:], wall_d[:, WSPLIT:],
                          max_dma_last_dim=4096)
        wgate = wp.tile([128, NGATE], BF16)
        nc.scalar.dma_start(wgate[:], wg_d[:], max_dma_last_dim=4096)
        zrep = big.tile([128, R], BF16)
        nc.scalar.dma_start(zrep[:], zr_d[:])
        ckt = wp.tile([128, 6], F32)
        nc.scalar.dma_start(ckt[:], ck_d[:])

        def wsl(name, p0, pn, c0, cn_):
            if name in _GOFF:
                o = _GOFF[name]
                return wgate[p0:p0 + pn, o + c0:o + c0 + cn_]
            o = _WOFF[name]
            return wall[p0:p0 + pn, o + c0:o + c0 + cn_]
        epsc = ckt[:, 0:1]
        g0b, g1b, b2c = ckt[:, 1:2], ckt[:, 2:3], ckt[:, 3:4]
        g2b = ckt[0:8, 4:5]

        # ---------------- persistent activation tiles ----------------
        cnT = big.tile([128, R], BF16)     # LayerNormed c (gamma/beta folded out)
        gh0 = big.tile([128, R], BF16)     # gate hidden 1 (= elu+1)
        gh1 = big.tile([128, R], BF16)
        eL = big.tile([8, R], BF16)        # exp(gate logits)
        coeffN = big.tile([8, R], BF16)    # softmax coeffs
        s0a = big.tile([128, R], BF16)     # layer0 out (= elu+1), feat 0..127
        s0b = big.tile([128, R], BF16)     # feat 128..255
        s1a = big.tile([128, R], BF16)
        s1b = big.tile([128, R], BF16)
        zs = [big.tile([128, R], BF16, name=f"zs{q}") for q in range(2)]
        cball = big.tile([128, E * R], BF16)   # per-expert coeff broadcast
        cbz = [big.tile([128, R], BF16, name=f"cbz{q}") for q in range(2)]
        cbe16 = big.tile([128, R], BF16)
        cb = [cball[:, e * R:(e + 1) * R] for e in range(E)]
        otb = big.tile([128, NCH * ACTD], F32)

        # ---------------- stage A: LayerNorm stats ----------------
        mv8 = sp.tile([128, 16], F32, tag="mv8", bufs=1)
        for j in range(NCH):
            ct = ctall[:, 128 * j:128 * (j + 1)]
            stats = sp.tile([128, 6], F32, tag="st")
            nc.vector.bn_stats(stats[:], ct[:])
            nc.vector.bn_aggr(mv8[:, 2 * j:2 * j + 2], stats[:])
        var8 = AP(mv8[:].tensor, mv8[:].offset + 1, [list(mv8[:].ap[0]), [2, NCH]])
        lnv8 = sp.tile([128, NCH], F32, tag="sd", bufs=1)
        nc.scalar.activation(lnv8[:], var8, AF.Ln, bias=epsc[:])
        rstd8 = sp.tile([128, NCH], F32, tag="rs", bufs=1)
        nc.scalar.activation(rstd8[:], lnv8[:], AF.Exp, scale=-0.5)

        def ln_chunk(j):
            js = slice(128 * j, 128 * (j + 1))
            ct = ctall[:, js]
            y = sc.tile([128, 128], BF16, tag="y")
            nc.vector.tensor_scalar(y[:], ct[:], mv8[:, 2 * j:2 * j + 1],
                                    rstd8[:, j:j + 1], OP.subtract, OP.mult)
            yT = pm.tile([128, 128], BF16, tag="mm", name=f"tp{j}")
            nc.tensor.transpose(yT[:], y[:], wsl("i16", 0, 128, 0, 128))
            nc.scalar.copy(cnT[:, js], yT[:])

        # ---------------- stage B: gate + coeff broadcast ----------------
        # DRAM staging: quarter q occupies rows 8q..8q+8 of [32, BQ]
        cstage = dstage.tile([4 * 8, BQ], BF16)
        ctens = cstage.tensor

        def gate_mlp_layer(q, win, bvec, dst):
            qs = slice(BQ * q, BQ * (q + 1))
            pre = pm.tile([128, BQ], F32, tag="mm", name=f"{win}_{q}")
            if win == "g0":
                nc.tensor.matmul(pre[:], wsl("g0z", 0, 32, 0, 128),
                                 zrep[0:32, qs], start=True, stop=False)
                nc.tensor.matmul(pre[:], wsl("g0c", 0, 128, 0, 128),
                                 cnT[:, qs], start=False, stop=True)
            else:
                nc.tensor.matmul(pre[:], wsl("g1w", 0, 128, 0, 128),
                                 gh0[:, qs], start=True, stop=True)
            ee = er.tile([128, BQ], BF16, tag="eg")
            nc.scalar.activation(ee[:], pre[:], AF.Exp, bias=bvec[:])
            rr = er.tile([128, BQ], BF16, tag="rg")
            nc.vector.tensor_scalar(rr[:], pre[:], bvec[:], 0.0, OP.add, OP.max)
            nc.vector.scalar_tensor_tensor(dst[:, qs], ee[:], 1.0, rr[:],
                                           OP.min, OP.add)

        def gate_l2_quarter(q):
            qs = slice(BQ * q, BQ * (q + 1))
            pre2 = psm.tile([8, BQ], F32, tag="sm")
            nc.tensor.matmul(pre2[:], wsl("g2w", 0, 128, 0, 8), gh1[:, qs],
                             start=True, stop=True)
            nc.scalar.activation(eL[:, qs], pre2[:], AF.Exp, bias=g2b[:])
            sume8 = psm.tile([8, BQ], F32, tag="sm")
            nc.tensor.matmul(sume8[:], wsl("on88", 0, 8, 0, 8), eL[:, qs],
                             start=True, stop=True)
            rsum8 = sp.tile([8, BQ], F32, tag="rsm")
            nc.vector.reciprocal_approx_fast(rsum8[:], sume8[:])
            nc.vector.tensor_mul(coeffN[:, qs], eL[:, qs], rsum8[:])
            # stage quarter to a contiguous DRAM block, broadcast back with
            # 4KB-contiguous-source descriptors
            eng_main = nc.sync if q % 2 == 0 else nc.scalar
            eng_aux = nc.scalar if q % 2 == 0 else nc.sync
            qo = 8 * BQ * q
            eng_main.dma_start(AP(ctens, qo, [[BQ, 8], [1, BQ]]), coeffN[:, qs])
            if q >= 2:
                eng_main.dma_start(
                    AP(cball.tensor, BQ * q, [[E * R, 128], [R, E], [1, BQ]]),
                    AP(ctens, qo, [[0, 128], [1, E * BQ]]))
            if q >= 2:
                for qz in range(2):
                    eng_aux.dma_start(
                        cbz[qz][:, qs],
                        AP(ctens, qo + 4 * BQ * qz, [[BQ, 4], [0, 32], [1, BQ]]))
            eng_aux.dma_start(
                cbe16[:, qs],
                AP(ctens, qo, [[BQ, 8], [0, 16], [1, BQ]]))
            if q == 3:
                bs = slice(BG, 2 * BG)
                for qz in range(2):
                    nc.vector.tensor_mul(zs[qz][:, bs], zrep[:, bs],
                                         cbz[qz][:, bs])

        # ---------------- expert layers (per half) ----------------
        def elu_plus1(ps, dst, bs, tagsfx, fine=False):
            ee = er.tile([128, BG], BF16, tag="e" + tagsfx)
            nc.scalar.activation(ee[:], ps[:], AF.Exp)
            rr = er.tile([128, BG], BF16, tag="r" + tagsfx)
            nc.scalar.activation(rr[:], ps[:], AF.Relu)
            if fine:
                for u in range(2):
                    us = slice(BQ * u, BQ * (u + 1))
                    ds = slice(bs.start + BQ * u, bs.start + BQ * (u + 1))
                    nc.vector.scalar_tensor_tensor(dst[:, ds], ee[:, us], 1.0,
                                                   rr[:, us], OP.min, OP.add)
            else:
                nc.vector.scalar_tensor_tensor(dst[:, bs], ee[:], 1.0, rr[:],
                                               OP.min, OP.add)

        def expert_layer(h, wzn, whn, bias_off, srcs, tag, dsts,
                         pe_bcast=False, fine_tail=False):
            bs = slice(BG * h, BG * (h + 1))
            nkt = len(srcs)
            ps = [pm.tile([128, BG], F32, tag="mm", name=f"ps{tag}{h}_{mt}")
                  for mt in range(2)]
            for mt in range(2):
                nc.tensor.matmul(ps[mt][:],
                                 wsl("b01", 0, 8, bias_off + 128 * mt, 128),
                                 coeffN[:, bs], start=True, stop=False)
            if pe_bcast:
                # bootstrap h0: broadcast coeffs on the PE (one-hot matmul
                # into PSUM) instead of waiting for the DMA round-trip; also
                # build zs from a PE-broadcast selector.
                for qz in range(2):
                    czP = pm.tile([128, BG], F32, tag="cbp", bufs=2,
                                  name=f"czp{qz}")
                    nc.tensor.matmul(czP[:], wsl("sel8", 0, 8, 128 * qz, 128),
                                     coeffN[:, bs], start=True, stop=True)
                    nc.vector.tensor_mul(zs[qz][:, bs], zrep[:, bs], czP[:])
            for kt in range(nkt):
                e, srct = srcs[kt]
                t = sc.tile([128, BG], BF16, tag=tag, name=f"x{tag}{h}_{kt}")
                if pe_bcast:
                    cbP = pm.tile([128, BG], F32, tag="cbp", bufs=2,
                                  name=f"cbp{kt}")
                    nc.tensor.matmul(cbP[:], wsl("ohe", 0, 8, 128 * e, 128),
                                     coeffN[:, bs], start=True, stop=True)
                    nc.scalar.copy(cb[e][:, bs], cbP[:])
                nc.vector.tensor_mul(t[:], srct[:, bs], cb[e][:, bs])
                for mt in range(2):
                    nc.tensor.matmul(ps[mt][:],
                                     wsl(whn, 0, 128, 256 * kt + 128 * mt, 128),
                                     t[:, :], start=False, stop=False)
            for kt in range(2):
                for mt in range(2):
                    nc.tensor.matmul(ps[mt][:],
                                     wsl(wzn, 0, 128, 256 * kt + 128 * mt, 128),
                                     zs[kt][:, bs], start=False,
                                     stop=(kt == 1 and mt == 1))
            for mt in range(2):
                elu_plus1(ps[mt], dsts[mt], bs, tag, fine=fine_tail)

        def layer2(h, fine=False):
            bs = slice(BG * h, BG * (h + 1))
            per2 = pm.tile([128, BG], F32, tag="mm", name=f"l2_{h}")
            nu = 2 if fine else 1
            w = BG // nu
            for u in range(nu):
                us = slice(w * u, w * (u + 1))
                ds = slice(bs.start + w * u, bs.start + w * (u + 1))
                nc.tensor.matmul(per2[:, us], wsl("w2s", 0, 32, 0, 128),
                                 zrep[0:32, ds], start=True, stop=False)
                nc.tensor.matmul(per2[:, us], wsl("w2s", 0, 128, 128, 128),
                                 s1a[:, ds], start=False, stop=False)
                nc.tensor.matmul(per2[:, us], wsl("w2s", 0, 128, 256, 128),
                                 s1b[:, ds], start=False, stop=True)
                mixed = er.tile([128, w], BF16, tag="mx")
                nc.vector.scalar_tensor_tensor(mixed[:], per2[:, us], b2c[:],
                                               cbe16[:, ds], OP.add, OP.mult)
                for jj in range(w // 128):
                    r = (BG // 128) * h + (w // 128) * u + jj
                    op = pm.tile([128, ACTD], F32, tag="cbp", bufs=2)
                    nc.tensor.matmul(op[:], mixed[:, 128 * jj:128 * (jj + 1)],
                                     wsl("s2", 0, 128, 0, 16), start=True,
                                     stop=True)
                    nc.scalar.copy(otb[:, ACTD * r:ACTD * (r + 1)], op[:])
            # ship this half: partition p holds out rows 8p+4h..8p+4h+4
            if fine:
                nc.sync.dma_start(
                    AP(out_d.tensor, 64 * h, [[NCH * ACTD, 128], [1, 32]]),
                    otb[:, 64 * h:64 * h + 32])
                nc.sync.dma_start(
                    AP(out_d.tensor, 64 * h + 32, [[NCH * ACTD, 128], [1, 32]]),
                    otb[:, 64 * h + 32:64 * h + 64])
            else:
                nc.sync.dma_start(
                    AP(out_d.tensor, 64 * h, [[NCH * ACTD, 128], [1, 64]]),
                    otb[:, 64 * h:64 * h + 64])

        # ---------------- emission order (gate pipelined by quarter) --------
        ln_chunk(0)
        ln_chunk(1)
        gate_mlp_layer(0, "g0", g0b, gh0)
        ln_chunk(2)
        ln_chunk(3)
        gate_mlp_layer(1, "g0", g0b, gh0)
        gate_mlp_layer(0, "g1", g1b, gh1)
        ln_chunk(4)
        ln_chunk(5)
        gate_mlp_layer(2, "g0", g0b, gh0)
        gate_mlp_layer(1, "g1", g1b, gh1)
        gate_l2_quarter(0)
        ln_chunk(6)
        ln_chunk(7)
        gate_mlp_layer(3, "g0", g0b, gh0)
        gate_mlp_layer(2, "g1", g1b, gh1)
        gate_l2_quarter(1)
        gate_mlp_layer(3, "g1", g1b, gh1)
        gate_l2_quarter(2)
        gate_l2_quarter(3)
        expert_layer(0, "w0z", "w0c", 0,
                     [(e, cnT) for e in range(E)], "sc0", (s0a, s0b),
                     pe_bcast=True)
        expert_layer(1, "w0z", "w0c", 0,
                     [(e, cnT) for e in range(E)], "sc0", (s0a, s0b))
        expert_layer(0, "w1z", "w1h", 256,
                     [(e, t) for e in range(E) for t in (s0a, s0b)],
                     "sc1", (s1a, s1b))
        layer2(0)
        expert_layer(1, "w1z", "w1h", 256,
                     [(e, t) for e in range(E) for t in (s0a, s0b)],
                     "sc1", (s1a, s1b), fine_tail=True)
        layer2(1, fine=True)

    nc.compile()
    return nc


def _host_prep(inputs):
    f = lambda a: np.ascontiguousarray(np.asarray(a, dtype=np.float32))
    w0, b0 = f(inputs["w0"]), f(inputs["b0"])
    w1, b1 = f(inputs["w1"]), f(inputs["b1"])
    w2, b2 = f(inputs["w2"]), f(inputs["b2"])
    g0w, g0b = f(inputs["g0w"]), f(inputs["g0b"])
    g1w, g1b = f(inputs["g1w"]), f(inputs["g1b"])
    g2w, g2b = f(inputs["g2w"]), f(inputs["g2b"])
    ln_g, ln_b = f(inputs["ln_g"]), f(inputs["ln_b"])

    # fold LayerNorm gamma/beta into the c-consuming weights and biases:
    # on-chip cn = (c - mu) * rstd; true cn = cn_core*gamma + beta.
    w0c = w0[:, LATENT:, :]                       # [E, CIN, HID]
    w0c_g = w0c * ln_g[None, :, None]
    b0_f = b0 + np.einsum("f,efo->eo", ln_b, w0c)
    g0c = g0w[LATENT:]                            # [CIN, GH]
    g0c_g = g0c * ln_g[:, None]
    g0b_f = g0b + ln_b @ g0c

    def ksb(wstk, nkt, m):   # [nkt*128, m] -> [128, nkt*m]
        return np.ascontiguousarray(
            wstk.reshape(nkt, 128, m).transpose(1, 0, 2).reshape(128, nkt * m))

    wall = np.zeros((128, NWALL), np.float32)
    wgate = np.zeros((128, NGATE), np.float32)
    def put(name, arr):
        if name in _GOFF:
            o = _GOFF[name]
            wgate[:arr.shape[0], o:o + arr.shape[1]] = arr
        else:
            o = _WOFF[name]
            wall[:arr.shape[0], o:o + arr.shape[1]] = arr

    put("w0z", ksb(w0[:, :LATENT, :].reshape(E * LATENT, HID), 2, HID))
    put("w0c", ksb(w0c_g.reshape(E * CIN, HID), 8, HID))
    put("w1z", ksb(w1[:, :LATENT, :].reshape(E * LATENT, HID), 2, HID))
    put("w1h", ksb(w1[:, LATENT:, :].reshape(E * HID, HID), 16, HID))
    w2stk = w2.transpose(1, 0, 2).reshape(INTER, E * ACTD)   # [288, 128]
    w2s = np.zeros((128, 384), np.float32)
    w2s[:32, 0:128] = w2stk[0:32]
    w2s[:, 128:256] = w2stk[32:160]
    w2s[:, 256:384] = w2stk[160:288]
    put("w2s", w2s)
    put("s2", np.tile(np.eye(ACTD, dtype=np.float32), (E, 1)))
    put("g0z", g0w[:LATENT])
    put("g0c", g0c_g)
    put("g1w", g1w)
    put("g2w", g2w)
    b1f = b1 - w1[:, LATENT:, :].sum(axis=1)
    put("b01", np.concatenate([b0_f, b1f], axis=1))
    put("on88", np.ones((8, 8), np.float32))
    put("i16", np.eye(128, dtype=np.float32))
    ohe = np.zeros((8, 8 * 128), np.float32)
    for e in range(8):
        ohe[e, 128 * e:128 * (e + 1)] = 1.0
    put("ohe", ohe)
    sel8 = np.zeros((8, 256), np.float32)
    for qz in range(2):
        for j in range(4):
            sel8[4 * qz + j, 128 * qz + 32 * j:128 * qz + 32 * (j + 1)] = 1.0
    put("sel8", sel8)

    b2f = b2 - w2[:, LATENT:, :].sum(axis=1)                 # [8,16]
    consts = np.zeros((128, 6), np.float32)
    consts[:, 0] = LN_EPS
    consts[:, 1] = g0b_f
    consts[:, 2] = g1b - g1w.sum(0)
    consts[:, 3] = b2f.reshape(128)
    consts[:8, 4] = (g2b - g2w.sum(0))
    return {"wall": wall.astype(NP_BF16), "wgate": wgate.astype(NP_BF16),
            "consts": consts}


def make_in_maps(inputs):
    wmap = _host_prep(inputs)
    z = np.ascontiguousarray(np.asarray(inputs["z"], dtype=np.float32))
    c = np.ascontiguousarray(np.asarray(inputs["c"], dtype=np.float32))
    # on-chip batch order: i = 128*r + p  <->  original row b = 8p + r
    ii = np.arange(R)
    perm = 8 * (ii % 128) + ii // 128
    in_maps = []
    for i in range(N_CORES):
        m = dict(wmap)
        zsh = z[i * R:(i + 1) * R]
        m["zrep"] = np.ascontiguousarray(np.tile(zsh.T[:, perm], (4, 1))).astype(NP_BF16)
        csh = c[i * R:(i + 1) * R]
        # partition p <- rows 8p..8p+8 (contiguous 4KB lines)
        m["cperm"] = np.ascontiguousarray(csh.reshape(128, NCH * CIN)).astype(NP_BF16)
        in_maps.append(m)
    return in_maps


def kernel(**inputs):
    if "nc" not in _CACHE:
        _CACHE["nc"] = _build_program()
    nc = _CACHE["nc"]
    in_maps = make_in_maps(inputs)
    res = bass_utils.run_bass_kernel_spmd(nc, in_maps, core_ids=list(range(N_CORES)))
    return np.concatenate([res.results[i]["out"] for i in range(N_CORES)], axis=0)
